# revision 9
# baseline (speedup 1.0000x reference)
"""BEVFeatureAggregation Trainium2 kernel.

Math: out[b,n,o] = inst[b,n,o] + b_proj[o]
                 + sum_c W_proj[o,c] * bilinear_sample(bev_map[b], anchor[b,n])[c]

Strategy (8 NeuronCores, core = batch*2 + anchor-half, 5000 anchors each):
  * anchors are concentrated in a tiny window of the 200x400 BEV map; the
    host computes the bounding box of all touched bilinear corners and ships
    only that subregion (C x R*K, zero-padded to C x RKP) per core.
  * device projects the subregion first:  S'[px,o] = sum_c sub[c,px]*W[o,c]
    (tiny fp32 matmul), so sampling directly produces output features.
  * bilinear sampling of 5000 anchors == dense matmul against a sparse
    weight matrix wb[px, n] (4 nonzeros per column: the corner weights).
    wb is host-built, split hi/lo in bf16 (hi + lo represents the fp32
    weight to ~2^-18), and S' is split the same way.  Three bf16 matmul
    passes (hi*hi + lo*hi + hi*lo) give fp32-grade accuracy at 1 PE
    cycle/row instead of fp32's 4.
  * the projection bias rides along as one extra "pixel" row: S'[BIAS,:] =
    b_proj with wb[BIAS,:] = 1.
  * epilogue fuses the residual add of instance_feature (transposed on
    host) while copying PSUM out; output is written (C, 5000) per core and
    transposed back on host.
"""

import numpy as np
import ml_dtypes

import concourse.bass as bass
import concourse.mybir as mybir
import concourse.tile as tile
from concourse.bass_utils import run_bass_kernel_spmd

# ---------------------------------------------------------------- constants
XMIN, XMAX, YMIN, YMAX = -80.0, 120.0, -40.0, 40.0
EPS = 1e-6
B, N, C, H, W = 4, 10000, 256, 200, 400
NCORES = 8
NPC = B * N // NCORES          # anchors per core
RKP_DEFAULT = 512              # padded pixel-slot count (multiple of 128)
RKP_MAX = 4096                 # beyond this fall back to host compute
F32 = mybir.dt.float32
BF16 = mybir.dt.bfloat16
NPBF16 = ml_dtypes.bfloat16

TRACE = False                  # set by test harness for profiling runs
LAST_RESULT = None             # BassKernelResults of the last device run

# --------------------------------------------------- walrus 1-wait workaround
# This container's walrus rejects >1 sem wait per instruction ("Too many
# sync wait commands").  Spread extra waits onto same-engine NoOps.

_MAXW = 1
_ctr = [0]


def _patched_drain_and_barrier(self, tick_clock, wait_clock):
    nc = self.nc
    probe = nc.sync.nop(hint="drain_wait_spread", nofuse=True)
    wait_clock.add_sem_waits(
        probe.ins, tile.ScopedClock({None: tick_clock.global_clock})
    )
    waits = list(probe.ins.sync_info.on_wait or [])
    if len(waits) > _MAXW:
        probe.ins.sync_info.on_wait = waits[:_MAXW]
        rest = waits[_MAXW:]
        while rest:
            chunk, rest = rest[:_MAXW], rest[_MAXW:]
            nxt = nc.sync.nop(hint="drain_wait_spread", nofuse=True)
            if nxt.ins.sync_info is None:
                nxt.ins.sync_info = mybir.SyncInfo(on_wait=chunk, on_update=[])
            else:
                nxt.ins.sync_info.on_wait = chunk
    nc.sync.drain()
    nc.all_engine_barrier()
    assert self.sems is not None
    popped = nc._tile_sem_poison_stack.pop()
    assert popped is self._sem_poison
    nc.clear_and_free_semaphores(list(self.sems.allocated().values()))
    nc.all_engine_barrier()


tile.TileContext._drain_and_barrier = _patched_drain_and_barrier


def _split_multiwait(nc):
    for f in nc.m.functions:
        for b in f.blocks:
            insts = list(b.instructions)
            out = []
            changed = False
            for inst in insts:
                si = inst.sync_info
                waits = list(si.on_wait) if (si and si.on_wait) else []
                if len(waits) > _MAXW:
                    changed = True
                    extra, keep = waits[:-_MAXW], waits[-_MAXW:]
                    si.on_wait = keep
                    inst.sync_info = si
                    for w in extra:
                        _ctr[0] += 1
                        nop = mybir.InstNoOp(
                            name=f"wsplit_{_ctr[0]}", ins=[], outs=[]
                        )
                        nop.engine = inst.engine
                        nop.sync_info = mybir.SyncInfo(on_wait=[w], on_update=[])
                        out.append(nop)
                out.append(inst)
            if changed:
                cur = b.instructions
                while len(cur):
                    cur.pop()
                for inst in out:
                    b.add_instruction(inst)


# ------------------------------------------------------------ device program
_programs = {}


def _n_tiles(width, step=512):
    offs = []
    t0 = 0
    while t0 < width:
        offs.append((t0, min(step, width - t0)))
        t0 += step
    return offs


def _build_program(rkp):
    kc = rkp // 128
    nc = bass.Bass()
    bev = nc.declare_dram_parameter("bev_sub", [C, rkp], F32, isOutput=False)
    wpt = nc.declare_dram_parameter("wproj_t", [C, C], F32, isOutput=False)
    bph = nc.declare_dram_parameter("bproj_hi", [1, C], BF16, isOutput=False)
    bpl = nc.declare_dram_parameter("bproj_lo", [1, C], BF16, isOutput=False)
    wbh = nc.declare_dram_parameter("wb_hi", [rkp, NPC], BF16, isOutput=False)
    wbl = nc.declare_dram_parameter("wb_lo", [rkp, NPC], BF16, isOutput=False)
    ins = nc.declare_dram_parameter("inst_t", [C, NPC], F32, isOutput=False)
    out = nc.declare_dram_parameter("out_t", [C, NPC], F32, isOutput=True)

    quarters = _n_tiles(NPC, 1280)

    with tile.TileContext(nc) as tc:
        with (
            tc.tile_pool(name="const", bufs=1) as constp,
            tc.tile_pool(name="work", bufs=2) as workp,
            tc.tile_pool(name="wb", bufs=2) as wbp,
            tc.tile_pool(name="io", bufs=3) as iop,
            tc.tile_pool(name="ps1", bufs=2, space="PSUM") as ps1p,
            tc.tile_pool(name="ps2", bufs=6, space="PSUM") as ps2p,
        ):
            # ---- phase 0: small loads
            wpt_sb = []
            for cc in range(2):
                t = constp.tile([128, C], F32, tag=f"wpt{cc}", name=f"wpt{cc}")
                nc.sync.dma_start(t[:], wpt[cc * 128:(cc + 1) * 128, :])
                wpt_sb.append(t)
            bev_sb = []
            for cc in range(2):
                t = constp.tile([128, rkp], F32, tag=f"bev{cc}", name=f"bev{cc}")
                nc.sync.dma_start(t[:], bev[cc * 128:(cc + 1) * 128, :])
                bev_sb.append(t)


            # ---- phase 1: project subregion, split hi/lo
            sp_hi, sp_lo = [], []
            for k in range(kc):
                ps = ps1p.tile([128, C], F32, tag="ps1", name=f"ps1_{k}")
                for cc in range(2):
                    nc.tensor.matmul(
                        ps[:],
                        lhsT=bev_sb[cc][:, k * 128:(k + 1) * 128],
                        rhs=wpt_sb[cc][:],
                        start=(cc == 0),
                        stop=(cc == 1),
                    )
                hi = constp.tile([128, C], BF16, tag=f"sph{k}", name=f"sph{k}")
                lo = constp.tile([128, C], BF16, tag=f"spl{k}", name=f"spl{k}")
                nc.vector.tensor_copy(hi[:], ps[:])
                tmp = workp.tile([128, C], F32, tag="split_tmp", name="split_tmp")
                nc.vector.tensor_copy(tmp[:], hi[:])
                nc.vector.tensor_sub(lo[:], ps[:], tmp[:])
                sp_hi.append(hi)
                sp_lo.append(lo)

            # bias row: S'[rkp-1] = b_proj (chunk kc-1, row 127); hi/lo split
            # is done on host, DMA'd over the matmul1-produced zeros.
            nc.sync.dma_start(sp_hi[kc - 1][127:128, :], bph[:, :])
            nc.sync.dma_start(sp_lo[kc - 1][127:128, :], bpl[:, :])

            # ---- phase 2: sampling matmul, 3 bf16 passes, fused epilogue
            passes = [(sp_hi, 0), (sp_lo, 0), (sp_hi, 1)]  # (lhsT set, wb idx)
            wb_dram = [wbh, wbl]
            for q0, qw in quarters:
                wb_sb = [[None] * kc for _ in range(2)]
                for ch in range(2):
                    for k in range(kc):
                        t = wbp.tile([128, 1280], BF16, tag=f"wb{ch}_{k}", name=f"wb{ch}_{k}")
                        nc.sync.dma_start(
                            t[:, :qw],
                            wb_dram[ch][k * 128:(k + 1) * 128, q0:q0 + qw],
                        )
                        wb_sb[ch][k] = t
                nts = _n_tiles(qw, 512)
                for oc in range(2):
                    pss = [ps2p.tile([128, tw], F32, tag="ps2", name=f"ps2_{ti}") for ti, (_, tw) in enumerate(nts)]
                    first = True
                    npass = len(passes) * kc
                    i = 0
                    for sp, ch in passes:
                        for k in range(kc):
                            i += 1
                            last = i == npass
                            lhs = sp[k][:, oc * 128:(oc + 1) * 128]
                            for ti, (t0, tw) in enumerate(nts):
                                nc.tensor.matmul(
                                    pss[ti][:],
                                    lhsT=lhs,
                                    rhs=wb_sb[ch][k][:, t0:t0 + tw],
                                    start=first,
                                    stop=last,
                                )
                            first = False
                    for ti, (t0, tw) in enumerate(nts):
                        it = iop.tile([128, 512], F32, tag="inst", name="inst_t_sb")
                        nc.sync.dma_start(
                            it[:, :tw],
                            ins[oc * 128:(oc + 1) * 128, q0 + t0:q0 + t0 + tw],
                        )
                        ot = iop.tile([128, 512], F32, tag="out", name="out_t_sb")
                        nc.vector.tensor_add(ot[:, :tw], pss[ti][:], it[:, :tw])
                        nc.sync.dma_start(
                            out[oc * 128:(oc + 1) * 128, q0 + t0:q0 + t0 + tw],
                            ot[:, :tw],
                        )

    return nc


def _get_program(rkp):
    if rkp not in _programs:
        _programs[rkp] = _build_program(rkp)
    return _programs[rkp]


# -------------------------------------------------------------- host prep
def _prep_core(anchor_bn, bev_b, inst_bn, rkp):
    """Build one core's input arrays.  anchor_bn: (NPC,>=2) f32,
    bev_b: (C,H,W) f32, inst_bn: (NPC,C) f32."""
    f = np.float32
    ax = anchor_bn[:, 0].astype(f)
    ay = anchor_bn[:, 1].astype(f)
    gx = (ax - f(XMIN)) / f(XMAX - XMIN + EPS) * f(2.0) - f(1.0)
    gy = (ay - f(YMIN)) / f(YMAX - YMIN + EPS) * f(2.0) - f(1.0)
    # module stacks [grid_y, grid_x]: width coord <- gy, height coord <- gx
    ix = (gy + f(1.0)) * f(0.5) * f(W - 1)
    iy = (gx + f(1.0)) * f(0.5) * f(H - 1)
    x0 = np.floor(ix)
    y0 = np.floor(iy)
    x1 = x0 + f(1.0)
    y1 = y0 + f(1.0)
    wx1 = ix - x0
    wx0 = f(1.0) - wx1
    wy1 = iy - y0
    wy0 = f(1.0) - wy1
    corners = []
    for xc, yc, w in ((x0, y0, wx0 * wy0), (x1, y0, wx1 * wy0),
                      (x0, y1, wx0 * wy1), (x1, y1, wx1 * wy1)):
        valid = (xc >= 0) & (xc <= W - 1) & (yc >= 0) & (yc <= H - 1)
        xi = np.clip(xc, 0, W - 1).astype(np.int64)
        yi = np.clip(yc, 0, H - 1).astype(np.int64)
        corners.append((xi, yi, valid, (w * valid.astype(f)).astype(f)))

    vx = np.concatenate([np.where(v, xi, -1) for xi, yi, v, w in corners])
    vy = np.concatenate([np.where(v, yi, -1) for xi, yi, v, w in corners])
    m = vx >= 0
    if m.any():
        xmin, xmax = int(vx[m].min()), int(vx[m].max())
        ymin, ymax = int(vy[m].min()), int(vy[m].max())
    else:
        xmin = xmax = ymin = ymax = 0
    R, K = ymax - ymin + 1, xmax - xmin + 1
    need = R * K + 2
    if rkp is None or need > rkp:
        return None, need  # caller picks a bigger rkp

    trash = rkp - 2
    bev_sub = np.zeros((C, rkp), f)
    bev_sub[:, :R * K] = bev_b[:, ymin:ymin + R, xmin:xmin + K].reshape(C, R * K)
    wb_hi = np.zeros((rkp, NPC), NPBF16)
    wb_lo = np.zeros((rkp, NPC), NPBF16)
    ar = np.arange(NPC)
    for xi, yi, valid, w in corners:
        px = np.where(valid, (yi - ymin) * K + (xi - xmin), trash)
        hi = w.astype(NPBF16)
        lo = (w - hi.astype(f)).astype(NPBF16)
        wb_hi[px, ar] = hi
        wb_lo[px, ar] = lo
    wb_hi[rkp - 1, :] = NPBF16(1.0)
    wb_lo[rkp - 1, :] = NPBF16(0.0)
    return {
        "bev_sub": bev_sub,
        "wb_hi": wb_hi,
        "wb_lo": wb_lo,
        "inst_t": np.ascontiguousarray(inst_bn.astype(f).T),
    }, need


def _host_fallback(instance_feature, anchor, bev_map, W_proj, b_proj):
    """Exact numpy computation; only for pathological inputs whose bbox
    exceeds RKP_MAX."""
    f = np.float32
    out = np.empty((B, N, C), f)
    for b in range(B):
        ax = anchor[b, :, 0].astype(f)
        ay = anchor[b, :, 1].astype(f)
        gx = (ax - f(XMIN)) / f(XMAX - XMIN + EPS) * f(2.0) - f(1.0)
        gy = (ay - f(YMIN)) / f(YMAX - YMIN + EPS) * f(2.0) - f(1.0)
        ix = (gy + f(1.0)) * f(0.5) * f(W - 1)
        iy = (gx + f(1.0)) * f(0.5) * f(H - 1)
        x0 = np.floor(ix)
        y0 = np.floor(iy)
        x1 = x0 + f(1.0)
        y1 = y0 + f(1.0)
        wx1 = ix - x0
        wx0 = f(1.0) - wx1
        wy1 = iy - y0
        wy0 = f(1.0) - wy1
        acc = np.zeros((N, C), f)
        fm = bev_map[b].reshape(C, H * W)
        for xc, yc, w in ((x0, y0, wx0 * wy0), (x1, y0, wx1 * wy0),
                          (x0, y1, wx0 * wy1), (x1, y1, wx1 * wy1)):
            valid = (xc >= 0) & (xc <= W - 1) & (yc >= 0) & (yc <= H - 1)
            xi = np.clip(xc, 0, W - 1).astype(np.int64)
            yi = np.clip(yc, 0, H - 1).astype(np.int64)
            g = fm[:, yi * W + xi].T
            acc += g * (w * valid.astype(f))[:, None]
        out[b] = acc @ W_proj.T.astype(f) + b_proj.astype(f)
    return out + instance_feature.astype(f)


# ------------------------------------------------------------------- kernel
def kernel(instance_feature, anchor, anchor_embed, bev_map, W_proj, b_proj):
    global LAST_RESULT
    f = np.float32
    instance_feature = np.asarray(instance_feature)
    anchor = np.asarray(anchor)
    bev_map = np.asarray(bev_map)
    W_proj = np.asarray(W_proj)
    b_proj = np.asarray(b_proj)

    # per-core host prep; grow rkp if the touched bbox is unusually large
    rkp = RKP_DEFAULT
    while True:
        maps, needs = [], []
        ok = True
        for core in range(NCORES):
            b, half = core // 2, core % 2
            sl = slice(half * NPC, (half + 1) * NPC)
            m, need = _prep_core(anchor[b, sl], bev_map[b],
                                 instance_feature[b, sl], rkp)
            needs.append(need)
            if m is None:
                ok = False
                break
            maps.append(m)
        if ok:
            break
        need = max(needs)
        if need > RKP_MAX:
            return _host_fallback(instance_feature, anchor, bev_map,
                                  W_proj, b_proj)
        rkp = -(-need // 128) * 128

    wpt = np.ascontiguousarray(W_proj.astype(f).T)
    bpf = b_proj.astype(f).reshape(1, C)
    bph = bpf.astype(NPBF16)
    bpl = (bpf - bph.astype(f)).astype(NPBF16)
    for m in maps:
        m["wproj_t"] = wpt
        m["bproj_hi"] = bph
        m["bproj_lo"] = bpl

    nc = _get_program(rkp)
    if not getattr(nc, "_wsplit_done", False):
        _split_multiwait(nc)
        nc._wsplit_done = True
    res = run_bass_kernel_spmd(nc, maps, list(range(NCORES)), trace=TRACE)
    LAST_RESULT = res

    out = np.empty((B, N, C), f)
    for core in range(NCORES):
        b, half = core // 2, core % 2
        sl = slice(half * NPC, (half + 1) * NPC)
        out[b, sl] = res.results[core]["out_t"].T
    return out


# revision 14
# speedup vs baseline: 1.2499x; 1.2499x over previous
"""BEVFeatureAggregation Trainium2 kernel.

Math: out[b,n,o] = inst[b,n,o] + b_proj[o]
                 + sum_c W_proj[o,c] * bilinear_sample(bev_map[b], anchor[b,n])[c]

Strategy (8 NeuronCores, core = batch*2 + anchor-half, 5000 anchors each):
  * anchors concentrate in a tiny window of the 200x400 BEV map; the host
    computes the bounding box (R rows x K cols) of all touched bilinear
    corners and ships only that subregion (C x R*K, zero-padded) per core.
  * the device projects the subregion first:  S'[px,o] = sum_c sub[c,px] *
    W_proj[o,c]  (small fp32 matmuls producing per-row-pair tiles), so
    sampling directly produces output features.
  * the host sorts anchors by their bilinear row y0 (un-permuting on the
    way out).  All 4 corners of an anchor with row y0 live in the 2*K-pixel
    window [y0*K, y0*K+2K) of the row-major subregion, so each sorted
    group's sampling is a dense matmul with contraction only over that
    window (<=128 typically) instead of the whole R*K bbox:
        out_T[o, n] = sum_px S'pair[px, o] * wb[px, n]
    wb (<=128 x NSLOT) holds the 4 bilinear corner weights per column.
  * weights and S' are split hi/lo in bf16 (hi+lo carries ~2^-18 relative
    precision); three bf16 matmul passes (Shi*Whi + Shi*Wlo + Slo*Whi)
    give fp32-grade accuracy at 1 PE cycle/row instead of fp32's 4.
  * epilogue fuses the residual add (instance_feature + b_proj, transposed
    and permuted on host) while copying PSUM out; host transposes /
    un-permutes the (C, NSLOT) result.
  * dummy matmuls keep the PE HAM clock warm while the initial DMAs land.

All 8 cores run one SPMD program whose loop structure (subtile layout) is
the per-row max across cores; it is rebuilt (and the NEFF recompiled) when
that structure changes, and cached for repeated calls with the same
structure.
"""

import numpy as np
import ml_dtypes

import concourse.bass as bass
import concourse.mybir as mybir
import concourse.tile as tile
from concourse.bass_utils import run_bass_kernel_spmd

# ---------------------------------------------------------------- constants
XMIN, XMAX, YMIN, YMAX = -80.0, 120.0, -40.0, 40.0
EPS = 1e-6
B, N, C, H, W = 4, 10000, 256, 200, 400
NCORES = 8
NPC = B * N // NCORES          # anchors per core
RK_MAX = 4096                  # bbox cap; beyond this fall back to host
SUBTILE = 512                  # max psum free width
WARMUP_MM = 12                 # dummy matmuls to keep the PE HAM-warm
F32 = mybir.dt.float32
BF16 = mybir.dt.bfloat16
NPBF16 = ml_dtypes.bfloat16

TRACE = False                  # set by test harness for profiling runs
LAST_RESULT = None             # BassKernelResults of the last device run

# --------------------------------------------------- walrus 1-wait workaround
# This container's walrus rejects >1 sem wait per instruction ("Too many
# sync wait commands").  Spread extra waits onto same-engine NoOps.

_MAXW = 1
_ctr = [0]


def _patched_drain_and_barrier(self, tick_clock, wait_clock):
    nc = self.nc
    probe = nc.sync.nop(hint="drain_wait_spread", nofuse=True)
    wait_clock.add_sem_waits(
        probe.ins, tile.ScopedClock({None: tick_clock.global_clock})
    )
    waits = list(probe.ins.sync_info.on_wait or [])
    if len(waits) > _MAXW:
        probe.ins.sync_info.on_wait = waits[:_MAXW]
        rest = waits[_MAXW:]
        while rest:
            chunk, rest = rest[:_MAXW], rest[_MAXW:]
            nxt = nc.sync.nop(hint="drain_wait_spread", nofuse=True)
            if nxt.ins.sync_info is None:
                nxt.ins.sync_info = mybir.SyncInfo(on_wait=chunk, on_update=[])
            else:
                nxt.ins.sync_info.on_wait = chunk
    nc.sync.drain()
    nc.all_engine_barrier()
    assert self.sems is not None
    popped = nc._tile_sem_poison_stack.pop()
    assert popped is self._sem_poison
    nc.clear_and_free_semaphores(list(self.sems.allocated().values()))
    nc.all_engine_barrier()


tile.TileContext._drain_and_barrier = _patched_drain_and_barrier


def _split_multiwait(nc):
    for f in nc.m.functions:
        for b in f.blocks:
            insts = list(b.instructions)
            out = []
            changed = False
            for inst in insts:
                si = inst.sync_info
                waits = list(si.on_wait) if (si and si.on_wait) else []
                if len(waits) > _MAXW:
                    changed = True
                    extra, keep = waits[:-_MAXW], waits[-_MAXW:]
                    si.on_wait = keep
                    inst.sync_info = si
                    for w in extra:
                        _ctr[0] += 1
                        nop = mybir.InstNoOp(
                            name=f"wsplit_{_ctr[0]}", ins=[], outs=[]
                        )
                        nop.engine = inst.engine
                        nop.sync_info = mybir.SyncInfo(on_wait=[w], on_update=[])
                        out.append(nop)
                out.append(inst)
            if changed:
                cur = b.instructions
                while len(cur):
                    cur.pop()
                for inst in out:
                    b.add_instruction(inst)


# ------------------------------------------------------------ device program
# structure = (rkp, Kw, ws, kch, n_pairs, nslot, subtiles); subtiles is a
# tuple of (pair_idx, col_offset, width).
_programs = {}


def _build_program(structure):
    rkp, Kw, ws, kch, n_pairs, nslot, subtiles = structure
    nc = bass.Bass()
    bev = nc.declare_dram_parameter("bev_sub", [C, rkp], F32, isOutput=False)
    wpt = nc.declare_dram_parameter("wproj_t", [C, C], F32, isOutput=False)
    wbh = nc.declare_dram_parameter("wb_hi", [kch * 128, nslot], BF16,
                                    isOutput=False)
    wbl = nc.declare_dram_parameter("wb_lo", [kch * 128, nslot], BF16,
                                    isOutput=False)
    ins = nc.declare_dram_parameter("instb_t", [C, nslot], F32, isOutput=False)
    out = nc.declare_dram_parameter("out_t", [C, nslot], F32, isOutput=True)

    with tile.TileContext(nc) as tc:
        with (
            tc.tile_pool(name="const", bufs=1) as constp,
            tc.tile_pool(name="io", bufs=4) as iop,
            tc.tile_pool(name="ps1", bufs=2, space="PSUM") as ps1p,
            tc.tile_pool(name="ps2", bufs=6, space="PSUM") as ps2p,
        ):
            # ---- input DMAs (issue order matters: bev first for matmul1)
            bev_sb = []
            for cc in range(2):
                t = constp.tile([128, rkp], F32, tag=f"bev{cc}", name=f"bev{cc}")
                nc.sync.dma_start(t[:], bev[cc * 128:(cc + 1) * 128, :])
                bev_sb.append(t)
            wpt_sb = []
            for cc in range(2):
                t = constp.tile([128, C], F32, tag=f"wpt{cc}", name=f"wpt{cc}")
                nc.sync.dma_start(t[:], wpt[cc * 128:(cc + 1) * 128, :])
                wpt_sb.append(t)
            wb_sb = []
            for ci, src in enumerate((wbh, wbl)):
                chunks = []
                for ch in range(kch):
                    t = constp.tile([128, nslot], BF16, tag=f"wb{ci}_{ch}",
                                    name=f"wb{ci}_{ch}")
                    nc.sync.dma_start(t[:], src[ch * 128:(ch + 1) * 128, :])
                    chunks.append(t)
                wb_sb.append(chunks)
            inst_sb = []
            for oc in range(2):
                t = constp.tile([128, nslot], F32, tag=f"instb{oc}",
                                name=f"instb{oc}")
                nc.sync.dma_start(t[:], ins[oc * 128:(oc + 1) * 128, :])
                inst_sb.append(t)

            # ---- PE warmup: keep the HAM clock hot while DMAs land
            wu = constp.tile([128, 512], BF16, tag="warm", name="warm")
            nc.vector.memset(wu[:], 0.0)
            wups = ps2p.tile([128, SUBTILE], F32, tag="ps2", name="wups")
            for _ in range(WARMUP_MM):
                nc.tensor.matmul(wups[:], lhsT=wu[:, 0:128], rhs=wu[:],
                                 start=True, stop=True)

            # ---- phase 1: project row-pair windows, split hi/lo
            # pair r covers subregion pixels [r*Kw, r*Kw + ws)
            sp_hi, sp_lo = [], []
            for r in range(n_pairs):
                his, los = [], []
                for ch in range(kch):
                    p0 = r * Kw + ch * 128
                    pw = max(0, min(128, ws - ch * 128, rkp - p0))
                    if pw == 0:
                        his.append((None, 0))
                        los.append((None, 0))
                        continue
                    ps = ps1p.tile([128, C], F32, tag="ps1", name=f"ps1_{r}_{ch}")
                    for cc in range(2):
                        nc.tensor.matmul(
                            ps[0:pw, :],
                            lhsT=bev_sb[cc][:, p0:p0 + pw],
                            rhs=wpt_sb[cc][:],
                            start=(cc == 0),
                            stop=(cc == 1),
                        )
                    hi = constp.tile([128, C], BF16, tag=f"sph{r}_{ch}",
                                     name=f"sph{r}_{ch}")
                    lo = constp.tile([128, C], BF16, tag=f"spl{r}_{ch}",
                                     name=f"spl{r}_{ch}")
                    nc.vector.tensor_copy(hi[0:pw, :], ps[0:pw, :])
                    tmp = iop.tile([128, C], F32, tag="split_tmp",
                                   name="split_tmp")
                    nc.vector.tensor_copy(tmp[0:pw, :], hi[0:pw, :])
                    nc.vector.tensor_sub(lo[0:pw, :], ps[0:pw, :], tmp[0:pw, :])
                    his.append((hi, pw))
                    los.append((lo, pw))
                sp_hi.append(his)
                sp_lo.append(los)

            # ---- phase 2: sampling matmuls + fused residual epilogue
            # pass order keeps the stationary operand repeated: hi, hi, lo
            for oc in range(2):
                for (r, c0, tw) in subtiles:
                    ps = ps2p.tile([128, SUBTILE], F32, tag="ps2",
                                   name=f"ps2_{oc}_{c0}")
                    mms = []
                    for sp, wbi in ((sp_hi, 0), (sp_hi, 1), (sp_lo, 0)):
                        for ch in range(kch):
                            t, pw = sp[r][ch]
                            if pw:
                                mms.append((t, pw, wbi, ch))
                    for i, (t, pw, wbi, ch) in enumerate(mms):
                        nc.tensor.matmul(
                            ps[:, 0:tw],
                            lhsT=t[0:pw, oc * 128:(oc + 1) * 128],
                            rhs=wb_sb[wbi][ch][0:pw, c0:c0 + tw],
                            start=(i == 0),
                            stop=(i == len(mms) - 1),
                        )
                    ot = iop.tile([128, SUBTILE], F32, tag="out", name="out_sb")
                    nc.vector.tensor_add(
                        ot[:, 0:tw], ps[:, 0:tw], inst_sb[oc][:, c0:c0 + tw]
                    )
                    nc.sync.dma_start(
                        out[oc * 128:(oc + 1) * 128, c0:c0 + tw], ot[:, 0:tw]
                    )

    return nc


def _get_program(structure):
    if structure not in _programs:
        nc = _build_program(structure)
        _split_multiwait(nc)
        nc._wsplit_done = True
        _programs[structure] = nc
    return _programs[structure]


# -------------------------------------------------------------- host prep
def _corners(anchor_bn):
    f = np.float32
    ax = anchor_bn[:, 0].astype(f)
    ay = anchor_bn[:, 1].astype(f)
    gx = (ax - f(XMIN)) / f(XMAX - XMIN + EPS) * f(2.0) - f(1.0)
    gy = (ay - f(YMIN)) / f(YMAX - YMIN + EPS) * f(2.0) - f(1.0)
    # module stacks [grid_y, grid_x]: width coord <- gy, height coord <- gx
    ix = (gy + f(1.0)) * f(0.5) * f(W - 1)
    iy = (gx + f(1.0)) * f(0.5) * f(H - 1)
    x0 = np.floor(ix)
    y0 = np.floor(iy)
    x1 = x0 + f(1.0)
    y1 = y0 + f(1.0)
    wx1 = ix - x0
    wx0 = f(1.0) - wx1
    wy1 = iy - y0
    wy0 = f(1.0) - wy1
    out = []
    for xc, yc, w in ((x0, y0, wx0 * wy0), (x1, y0, wx1 * wy0),
                      (x0, y1, wx0 * wy1), (x1, y1, wx1 * wy1)):
        valid = (xc >= 0) & (xc <= W - 1) & (yc >= 0) & (yc <= H - 1)
        xi = np.clip(xc, 0, W - 1).astype(np.int64)
        yi = np.clip(yc, 0, H - 1).astype(np.int64)
        out.append((xi, yi, valid, (w * valid.astype(f)).astype(f)))
    return out, y0


def _host_fallback(instance_feature, anchor, bev_map, W_proj, b_proj):
    """Exact numpy computation; only for pathological inputs whose bbox
    exceeds RK_MAX."""
    f = np.float32
    out = np.empty((B, N, C), f)
    for b in range(B):
        corners, _ = _corners(anchor[b])
        acc = np.zeros((N, C), f)
        fm = bev_map[b].reshape(C, H * W)
        for xi, yi, valid, w in corners:
            g = fm[:, yi * W + xi].T
            acc += g * w[:, None]
        out[b] = acc @ W_proj.T.astype(f) + b_proj.astype(f)
    return out + instance_feature.astype(f)


# ------------------------------------------------------------------- kernel
def kernel(instance_feature, anchor, anchor_embed, bev_map, W_proj, b_proj):
    global LAST_RESULT
    f = np.float32
    instance_feature = np.asarray(instance_feature)
    anchor = np.asarray(anchor)
    bev_map = np.asarray(bev_map)
    W_proj = np.asarray(W_proj)
    b_proj = np.asarray(b_proj)

    instb = instance_feature.astype(f) + b_proj.astype(f)[None, None, :]

    # ---- pass 1: per-core corner geometry
    cores = []
    for core in range(NCORES):
        b, half = core // 2, core % 2
        sl = slice(half * NPC, (half + 1) * NPC)
        corners, y0f = _corners(anchor[b, sl])
        vx = np.concatenate([np.where(v, xi, -1) for xi, yi, v, w in corners])
        vy = np.concatenate([np.where(v, yi, -1) for xi, yi, v, w in corners])
        m = vx >= 0
        if m.any():
            xmin, xmax = int(vx[m].min()), int(vx[m].max())
            ymin, ymax = int(vy[m].min()), int(vy[m].max())
        else:
            xmin = xmax = ymin = ymax = 0
        R, K = ymax - ymin + 1, xmax - xmin + 1
        if R * K > RK_MAX:
            return _host_fallback(instance_feature, anchor, bev_map,
                                  W_proj, b_proj)
        cores.append((corners, y0f, xmin, ymin, R, K))

    # ---- unified structure
    Kw = max(c[5] for c in cores)
    n_pairs = max(max(c[4] - 1, 1) for c in cores)
    ws = 2 * Kw
    kch = -(-ws // 128)
    rkp = 128 * -(-max((n_pairs - 1) * Kw + ws,
                       max(c[4] * Kw for c in cores)) // 128)
    if rkp > RK_MAX:
        return _host_fallback(instance_feature, anchor, bev_map,
                              W_proj, b_proj)

    y0ps = []
    counts = np.zeros((NCORES, n_pairs), np.int64)
    for core, (corners, y0f, xmin, ymin, R, K) in enumerate(cores):
        y0p = np.clip(y0f.astype(np.int64) - ymin, 0, max(R - 2, 0))
        y0p = np.minimum(y0p, n_pairs - 1)
        y0ps.append(y0p)
        counts[core] = np.bincount(y0p, minlength=n_pairs)
    cap = counts.max(axis=0)

    subtiles = []
    c0 = 0
    for r in range(n_pairs):
        left = int(cap[r])
        while left > 0:
            tw = min(SUBTILE, left)
            subtiles.append((r, c0, tw))
            c0 += tw
            left -= tw
    nslot = c0
    structure = (rkp, Kw, ws, kch, n_pairs, nslot, tuple(subtiles))

    # ---- pass 2: per-core arrays against the unified layout
    row_base = {}
    base = 0
    for r in range(n_pairs):
        row_base[r] = base
        base += int(cap[r])

    maps, perms = [], []
    wpt = np.ascontiguousarray(W_proj.astype(f).T)
    for core, (corners, y0f, xmin, ymin, R, K) in enumerate(cores):
        b, half = core // 2, core % 2
        sl = slice(half * NPC, (half + 1) * NPC)
        y0p = y0ps[core]
        # stable sort by pair row; columns are packed at each row's base
        order = np.argsort(y0p, kind="stable")
        cnt = counts[core]
        col_of = np.empty(NPC, np.int64)
        start = 0
        for r in range(n_pairs):
            end = start + int(cnt[r])
            col_of[order[start:end]] = row_base[r] + np.arange(end - start)
            start = end

        bev_sub = np.zeros((C, rkp), f)
        ke = min(xmin + Kw, W)
        bev_rows = bev_map[b][:, ymin:ymin + R, xmin:ke].astype(f)
        tmp = np.zeros((C, R, Kw), f)
        tmp[:, :, :ke - xmin] = bev_rows
        bev_sub[:, :R * Kw] = tmp.reshape(C, R * Kw)

        wb_hi = np.zeros((kch * 128, nslot), NPBF16)
        wb_lo = np.zeros((kch * 128, nslot), NPBF16)
        for xi, yi, valid, wgt in corners:
            px = (yi - ymin - y0p) * Kw + (xi - xmin)
            col = col_of[valid]
            pxv = px[valid]
            hi = wgt[valid].astype(NPBF16)
            lo = (wgt[valid] - hi.astype(f)).astype(NPBF16)
            wb_hi[pxv, col] = hi
            wb_lo[pxv, col] = lo

        instb_t = np.zeros((C, nslot), f)
        instb_t[:, col_of] = instb[b, sl].T

        maps.append({
            "bev_sub": bev_sub,
            "wb_hi": wb_hi,
            "wb_lo": wb_lo,
            "instb_t": instb_t,
            "wproj_t": wpt,
        })
        perms.append(col_of)

    nc = _get_program(structure)
    res = run_bass_kernel_spmd(nc, maps, list(range(NCORES)), trace=TRACE)
    LAST_RESULT = res

    out = np.empty((B, N, C), f)
    for core in range(NCORES):
        b, half = core // 2, core % 2
        sl = slice(half * NPC, (half + 1) * NPC)
        o = res.results[core]["out_t"]
        out[b, sl] = o[:, perms[core]].T
    return out


# revision 22
# speedup vs baseline: 1.3271x; 1.0618x over previous
"""BEVFeatureAggregation Trainium2 kernel.

Math: out[b,n,o] = inst[b,n,o] + b_proj[o]
                 + sum_c W_proj[o,c] * bilinear_sample(bev_map[b], anchor[b,n])[c]

Strategy (8 NeuronCores, core = batch*2 + anchor-half, 5000 anchors each):
  * anchors concentrate in a tiny window of the 200x400 BEV map; the host
    computes the bounding box (R rows x K cols) of all touched bilinear
    corners and ships only that subregion (C x R*K, zero-padded) per core.
  * the device projects the subregion first:  S'[px,o] = sum_c sub[c,px] *
    W_proj[o,c]  (small fp32 matmuls producing per-row-pair tiles), so
    sampling directly produces output features.
  * the host sorts anchors by their bilinear row y0 (un-permuting on the
    way out).  All 4 corners of an anchor with row y0 live in the 2*K-pixel
    window [y0*K, y0*K+2K) of the row-major subregion, so each sorted
    group's sampling is a dense matmul with contraction only over that
    window (<=128 typically) instead of the whole R*K bbox:
        out_T[o, n] = sum_px S'pair[px, o] * wb[px, n]
    wb (<=128 x NSLOT) holds the 4 bilinear corner weights per column.
  * weights and S' are split hi/lo in bf16 (hi+lo carries ~2^-18 relative
    precision); three bf16 matmul passes (Shi*Whi + Shi*Wlo + Slo*Whi)
    give fp32-grade accuracy at 1 PE cycle/row instead of fp32's 4.
  * epilogue fuses the residual add (instance_feature + b_proj, transposed
    and permuted on host) while copying PSUM out; host transposes /
    un-permutes the (C, NSLOT) result.
  * dummy matmuls keep the PE HAM clock warm while the initial DMAs land.

All 8 cores run one SPMD program whose loop structure (subtile layout) is
the per-row max across cores; it is rebuilt (and the NEFF recompiled) when
that structure changes, and cached for repeated calls with the same
structure.
"""

import numpy as np
import ml_dtypes

import concourse.bass as bass
import concourse.mybir as mybir
import concourse.tile as tile
from concourse.bass_utils import run_bass_kernel_spmd

# ---------------------------------------------------------------- constants
XMIN, XMAX, YMIN, YMAX = -80.0, 120.0, -40.0, 40.0
EPS = 1e-6
B, N, C, H, W = 4, 10000, 256, 200, 400
NCORES = 8
NPC = B * N // NCORES          # anchors per core
RK_MAX = 4096                  # bbox cap; beyond this fall back to host
SUBTILE = 512                  # max psum free width
WARMUP_MM = 10                 # dummy matmuls to keep the PE HAM-warm
BRIDGE_MM = 6                  # dummy matmuls between phase 1 and 2
DMA_PIECES = 4                 # split big loads so they complete in order
F32 = mybir.dt.float32
BF16 = mybir.dt.bfloat16
NPBF16 = ml_dtypes.bfloat16

TRACE = False                  # set by test harness for profiling runs
LAST_RESULT = None             # BassKernelResults of the last device run

# --------------------------------------------------- walrus 1-wait workaround
# This container's walrus rejects >1 sem wait per instruction ("Too many
# sync wait commands").  Spread extra waits onto same-engine NoOps.

_MAXW = 1
_ctr = [0]


def _patched_drain_and_barrier(self, tick_clock, wait_clock):
    nc = self.nc
    probe = nc.sync.nop(hint="drain_wait_spread", nofuse=True)
    wait_clock.add_sem_waits(
        probe.ins, tile.ScopedClock({None: tick_clock.global_clock})
    )
    waits = list(probe.ins.sync_info.on_wait or [])
    if len(waits) > _MAXW:
        probe.ins.sync_info.on_wait = waits[:_MAXW]
        rest = waits[_MAXW:]
        while rest:
            chunk, rest = rest[:_MAXW], rest[_MAXW:]
            nxt = nc.sync.nop(hint="drain_wait_spread", nofuse=True)
            if nxt.ins.sync_info is None:
                nxt.ins.sync_info = mybir.SyncInfo(on_wait=chunk, on_update=[])
            else:
                nxt.ins.sync_info.on_wait = chunk
    nc.sync.drain()
    nc.all_engine_barrier()
    assert self.sems is not None
    popped = nc._tile_sem_poison_stack.pop()
    assert popped is self._sem_poison
    nc.clear_and_free_semaphores(list(self.sems.allocated().values()))
    nc.all_engine_barrier()


tile.TileContext._drain_and_barrier = _patched_drain_and_barrier


def _split_multiwait(nc):
    for f in nc.m.functions:
        for b in f.blocks:
            insts = list(b.instructions)
            out = []
            changed = False
            for inst in insts:
                si = inst.sync_info
                waits = list(si.on_wait) if (si and si.on_wait) else []
                if len(waits) > _MAXW:
                    changed = True
                    extra, keep = waits[:-_MAXW], waits[-_MAXW:]
                    si.on_wait = keep
                    inst.sync_info = si
                    for w in extra:
                        _ctr[0] += 1
                        nop = mybir.InstNoOp(
                            name=f"wsplit_{_ctr[0]}", ins=[], outs=[]
                        )
                        nop.engine = inst.engine
                        nop.sync_info = mybir.SyncInfo(on_wait=[w], on_update=[])
                        out.append(nop)
                out.append(inst)
            if changed:
                cur = b.instructions
                while len(cur):
                    cur.pop()
                for inst in out:
                    b.add_instruction(inst)


# ------------------------------------------------------------ device program
# structure = (rkp, Kw, ws, kch, n_pairs, nslot, subtiles); subtiles is a
# tuple of (pair_idx, col_offset, width).
_programs = {}


def _build_program(structure):
    rkp, Kw, ws, kch, n_pairs, nslot, subtiles = structure
    nc = bass.Bass()
    bev = nc.declare_dram_parameter("bev_sub", [C, rkp], F32, isOutput=False)
    wpt = nc.declare_dram_parameter("wproj_t", [C, C], F32, isOutput=False)
    wbh = nc.declare_dram_parameter("wb_hi", [kch * 128, nslot], BF16,
                                    isOutput=False)
    wbl = nc.declare_dram_parameter("wb_lo", [kch * 128, nslot], BF16,
                                    isOutput=False)
    ins = nc.declare_dram_parameter("instb_t", [C, nslot], F32, isOutput=False)
    out = nc.declare_dram_parameter("out_t", [C, nslot], F32, isOutput=True)

    with tile.TileContext(nc) as tc:
        with (
            tc.tile_pool(name="const", bufs=1) as constp,
            tc.tile_pool(name="io", bufs=4) as iop,
            tc.tile_pool(name="ps", bufs=8, space="PSUM") as psp,
        ):
            # ---- PE warmup first: dummy matmuls on an uninitialized tile
            # (result never read) keep the HAM clock hot while DMAs land.
            wu = constp.tile([128, 512], BF16, tag="warm", name="warm")
            nc.gpsimd.memset(wu[:], 0.0)
            wups = psp.tile([128, SUBTILE], F32, tag="ps", name="wups")
            for _ in range(WARMUP_MM):
                nc.tensor.matmul(wups[:], lhsT=wu[:, 0:128], rhs=wu[:],
                                 start=True, stop=True)

            # ---- input DMAs; split into pieces so completion follows issue
            # order (queues round-robin): bev/wpt first, then wb, then inst.
            def _piece_dma(t, src, width, dtype_bytes, pieces):
                step = -(-width // pieces)
                s = 0
                while s < width:
                    e = min(s + step, width)
                    nc.sync.dma_start(t[:, s:e], src[:, s:e])
                    s = e

            bev_sb = []
            for cc in range(2):
                t = constp.tile([128, rkp], F32, tag=f"bev{cc}", name=f"bev{cc}")
                nc.sync.dma_start(t[:], bev[cc * 128:(cc + 1) * 128, :])
                bev_sb.append(t)
            wpt_sb = []
            for cc in range(2):
                t = constp.tile([128, C], F32, tag=f"wpt{cc}", name=f"wpt{cc}")
                nc.sync.dma_start(t[:], wpt[cc * 128:(cc + 1) * 128, :])
                wpt_sb.append(t)
            wb_sb = []
            for ci, src in enumerate((wbh, wbl)):
                chunks = []
                for ch in range(kch):
                    t = constp.tile([128, nslot], BF16, tag=f"wb{ci}_{ch}",
                                    name=f"wb{ci}_{ch}")
                    _piece_dma(t, src[ch * 128:(ch + 1) * 128, :], nslot, 2,
                               DMA_PIECES)
                    chunks.append(t)
                wb_sb.append(chunks)
            inst_sb = []
            for oc in range(2):
                t = constp.tile([128, nslot], F32, tag=f"instb{oc}",
                                name=f"instb{oc}")
                _piece_dma(t, ins[oc * 128:(oc + 1) * 128, :], nslot, 4,
                           DMA_PIECES)
                inst_sb.append(t)

            # ---- phase 1: project row-pair windows, split hi/lo
            # pair r covers subregion pixels [r*Kw, r*Kw + ws)
            sp_hi, sp_lo = [], []
            for r in range(n_pairs):
                his, los = [], []
                for ch in range(kch):
                    p0 = r * Kw + ch * 128
                    pw = max(0, min(128, ws - ch * 128, rkp - p0))
                    if pw == 0:
                        his.append((None, 0))
                        los.append((None, 0))
                        continue
                    ps = psp.tile([128, SUBTILE], F32, tag="ps",
                                  name=f"ps1_{r}_{ch}")
                    for cc in range(2):
                        nc.tensor.matmul(
                            ps[0:pw, 0:C],
                            lhsT=bev_sb[cc][:, p0:p0 + pw],
                            rhs=wpt_sb[cc][:],
                            start=(cc == 0),
                            stop=(cc == 1),
                        )
                    hi = constp.tile([128, C], BF16, tag=f"sph{r}_{ch}",
                                     name=f"sph{r}_{ch}")
                    lo = constp.tile([128, C], BF16, tag=f"spl{r}_{ch}",
                                     name=f"spl{r}_{ch}")
                    nc.vector.tensor_copy(hi[0:pw, 0:C], ps[0:pw, 0:C])
                    tmp = iop.tile([128, C], F32, tag="split_tmp",
                                   name="split_tmp")
                    nc.any.tensor_copy(tmp[0:pw, :], hi[0:pw, 0:C])
                    nc.vector.tensor_sub(lo[0:pw, 0:C], ps[0:pw, 0:C],
                                         tmp[0:pw, :])
                    his.append((hi, pw))
                    los.append((lo, pw))
                sp_hi.append(his)
                sp_lo.append(los)

            # bridge dummies: keep the PE busy while wb/inst DMAs land
            for _ in range(BRIDGE_MM):
                nc.tensor.matmul(wups[:], lhsT=wu[:, 0:128], rhs=wu[:],
                                 start=True, stop=True)

            # ---- phase 2: sampling matmuls + fused residual epilogue
            # pass order keeps the stationary operand repeated: hi, hi, lo
            for oc in range(2):
                for (r, c0, tw) in subtiles:
                    ps = psp.tile([128, SUBTILE], F32, tag="ps",
                                  name=f"ps2_{oc}_{c0}")
                    mms = []
                    for sp, wbi in ((sp_hi, 0), (sp_hi, 1), (sp_lo, 0)):
                        for ch in range(kch):
                            t, pw = sp[r][ch]
                            if pw:
                                mms.append((t, pw, wbi, ch))
                    for i, (t, pw, wbi, ch) in enumerate(mms):
                        nc.tensor.matmul(
                            ps[:, 0:tw],
                            lhsT=t[0:pw, oc * 128:(oc + 1) * 128],
                            rhs=wb_sb[wbi][ch][0:pw, c0:c0 + tw],
                            start=(i == 0),
                            stop=(i == len(mms) - 1),
                        )
                    ot = iop.tile([128, SUBTILE], F32, tag="out", name="out_sb")
                    nc.any.tensor_add(
                        ot[:, 0:tw], ps[:, 0:tw], inst_sb[oc][:, c0:c0 + tw]
                    )
                    nc.sync.dma_start(
                        out[oc * 128:(oc + 1) * 128, c0:c0 + tw], ot[:, 0:tw]
                    )

    return nc


def _get_program(structure):
    if structure not in _programs:
        nc = _build_program(structure)
        _split_multiwait(nc)
        nc._wsplit_done = True
        _programs[structure] = nc
    return _programs[structure]


# -------------------------------------------------------------- host prep
def _corners(anchor_bn):
    f = np.float32
    ax = anchor_bn[:, 0].astype(f)
    ay = anchor_bn[:, 1].astype(f)
    gx = (ax - f(XMIN)) / f(XMAX - XMIN + EPS) * f(2.0) - f(1.0)
    gy = (ay - f(YMIN)) / f(YMAX - YMIN + EPS) * f(2.0) - f(1.0)
    # module stacks [grid_y, grid_x]: width coord <- gy, height coord <- gx
    ix = (gy + f(1.0)) * f(0.5) * f(W - 1)
    iy = (gx + f(1.0)) * f(0.5) * f(H - 1)
    x0 = np.floor(ix)
    y0 = np.floor(iy)
    x1 = x0 + f(1.0)
    y1 = y0 + f(1.0)
    wx1 = ix - x0
    wx0 = f(1.0) - wx1
    wy1 = iy - y0
    wy0 = f(1.0) - wy1
    out = []
    for xc, yc, w in ((x0, y0, wx0 * wy0), (x1, y0, wx1 * wy0),
                      (x0, y1, wx0 * wy1), (x1, y1, wx1 * wy1)):
        valid = (xc >= 0) & (xc <= W - 1) & (yc >= 0) & (yc <= H - 1)
        xi = np.clip(xc, 0, W - 1).astype(np.int64)
        yi = np.clip(yc, 0, H - 1).astype(np.int64)
        out.append((xi, yi, valid, (w * valid.astype(f)).astype(f)))
    return out, y0


def _host_fallback(instance_feature, anchor, bev_map, W_proj, b_proj):
    """Exact numpy computation; only for pathological inputs whose bbox
    exceeds RK_MAX."""
    f = np.float32
    out = np.empty((B, N, C), f)
    for b in range(B):
        corners, _ = _corners(anchor[b])
        acc = np.zeros((N, C), f)
        fm = bev_map[b].reshape(C, H * W)
        for xi, yi, valid, w in corners:
            g = fm[:, yi * W + xi].T
            acc += g * w[:, None]
        out[b] = acc @ W_proj.T.astype(f) + b_proj.astype(f)
    return out + instance_feature.astype(f)


# ------------------------------------------------------------------- kernel
def kernel(instance_feature, anchor, anchor_embed, bev_map, W_proj, b_proj):
    global LAST_RESULT
    f = np.float32
    instance_feature = np.asarray(instance_feature)
    anchor = np.asarray(anchor)
    bev_map = np.asarray(bev_map)
    W_proj = np.asarray(W_proj)
    b_proj = np.asarray(b_proj)

    instb = instance_feature.astype(f) + b_proj.astype(f)[None, None, :]

    # ---- pass 1: per-core corner geometry
    cores = []
    for core in range(NCORES):
        b, half = core // 2, core % 2
        sl = slice(half * NPC, (half + 1) * NPC)
        corners, y0f = _corners(anchor[b, sl])
        vx = np.concatenate([np.where(v, xi, -1) for xi, yi, v, w in corners])
        vy = np.concatenate([np.where(v, yi, -1) for xi, yi, v, w in corners])
        m = vx >= 0
        if m.any():
            xmin, xmax = int(vx[m].min()), int(vx[m].max())
            ymin, ymax = int(vy[m].min()), int(vy[m].max())
        else:
            xmin = xmax = ymin = ymax = 0
        R, K = ymax - ymin + 1, xmax - xmin + 1
        if R * K > RK_MAX:
            return _host_fallback(instance_feature, anchor, bev_map,
                                  W_proj, b_proj)
        cores.append((corners, y0f, xmin, ymin, R, K))

    # ---- unified structure
    Kw = max(c[5] for c in cores)
    n_pairs = max(max(c[4] - 1, 1) for c in cores)
    ws = 2 * Kw
    kch = -(-ws // 128)
    rkp = 128 * -(-max((n_pairs - 1) * Kw + ws,
                       max(c[4] * Kw for c in cores)) // 128)
    if rkp > RK_MAX:
        return _host_fallback(instance_feature, anchor, bev_map,
                              W_proj, b_proj)

    y0ps = []
    counts = np.zeros((NCORES, n_pairs), np.int64)
    for core, (corners, y0f, xmin, ymin, R, K) in enumerate(cores):
        y0p = np.clip(y0f.astype(np.int64) - ymin, 0, max(R - 2, 0))
        y0p = np.minimum(y0p, n_pairs - 1)
        y0ps.append(y0p)
        counts[core] = np.bincount(y0p, minlength=n_pairs)
    cap = counts.max(axis=0)

    subtiles = []
    c0 = 0
    for r in range(n_pairs):
        left = int(cap[r])
        while left > 0:
            tw = min(SUBTILE, left)
            subtiles.append((r, c0, tw))
            c0 += tw
            left -= tw
    nslot = c0
    structure = (rkp, Kw, ws, kch, n_pairs, nslot, tuple(subtiles))

    # ---- pass 2: per-core arrays against the unified layout
    row_base = {}
    base = 0
    for r in range(n_pairs):
        row_base[r] = base
        base += int(cap[r])

    maps, perms = [], []
    wpt = np.ascontiguousarray(W_proj.astype(f).T)
    for core, (corners, y0f, xmin, ymin, R, K) in enumerate(cores):
        b, half = core // 2, core % 2
        sl = slice(half * NPC, (half + 1) * NPC)
        y0p = y0ps[core]
        # stable sort by pair row; columns are packed at each row's base
        order = np.argsort(y0p, kind="stable")
        cnt = counts[core]
        col_of = np.empty(NPC, np.int64)
        start = 0
        for r in range(n_pairs):
            end = start + int(cnt[r])
            col_of[order[start:end]] = row_base[r] + np.arange(end - start)
            start = end

        bev_sub = np.zeros((C, rkp), f)
        ke = min(xmin + Kw, W)
        bev_rows = bev_map[b][:, ymin:ymin + R, xmin:ke].astype(f)
        tmp = np.zeros((C, R, Kw), f)
        tmp[:, :, :ke - xmin] = bev_rows
        bev_sub[:, :R * Kw] = tmp.reshape(C, R * Kw)

        wb_hi = np.zeros((kch * 128, nslot), NPBF16)
        wb_lo = np.zeros((kch * 128, nslot), NPBF16)
        for xi, yi, valid, wgt in corners:
            px = (yi - ymin - y0p) * Kw + (xi - xmin)
            col = col_of[valid]
            pxv = px[valid]
            hi = wgt[valid].astype(NPBF16)
            lo = (wgt[valid] - hi.astype(f)).astype(NPBF16)
            wb_hi[pxv, col] = hi
            wb_lo[pxv, col] = lo

        instb_t = np.zeros((C, nslot), f)
        instb_t[:, col_of] = instb[b, sl].T

        maps.append({
            "bev_sub": bev_sub,
            "wb_hi": wb_hi,
            "wb_lo": wb_lo,
            "instb_t": instb_t,
            "wproj_t": wpt,
        })
        perms.append(col_of)

    nc = _get_program(structure)
    res = run_bass_kernel_spmd(nc, maps, list(range(NCORES)), trace=TRACE)
    LAST_RESULT = res

    out = np.empty((B, N, C), f)
    for core in range(NCORES):
        b, half = core // 2, core % 2
        sl = slice(half * NPC, (half + 1) * NPC)
        o = res.results[core]["out_t"]
        out[b, sl] = o[:, perms[core]].T
    return out
